# revision 29
# baseline (speedup 1.0000x reference)
"""KAN layer kernel for Trainium2, data-parallel over 8 NeuronCores.

Math: out[b,o] = sum_i comb_w[i,o] * (w1*x + w2*x^2 + w3*x^3 + edge_b)[b,i,o]
    = x @ W1 + x^2 @ W2 + x^3 @ W3 + bias
  where Wp[i,o] = edge_w[i,o,p] * comb_w[i,o],  bias[o] = sum_i comb_w[i,o]*edge_b[i,o].

Sharding: batch 8-way (1024 rows/core), weights replicated.

Precision: hybrid. The x and x^2 terms (2/3 of the contraction, small share
of the output error) run in fp8-e4m3 DoubleRow matmuls — each instruction
contracts 256 (a (W1,W2) weight pair against an (x, x^2) basis pair) so the
x/x^2 work needs half the instructions of bf16. The error-dominant x^3 term
stays bf16. All weights are pre-scaled by S=256 on the host so fp8 values
sit in e4m3's sweet spot; the PSUM result is descaled by 1/S during the
bias-add copy. Verified numerics: max rel err 1.13e-2 vs the fp32 reference
(gate is 2e-2). PSUM accumulates fp32 throughout.

Per core:
- x^T arrives as [128, 4, 1024] bf16 (partition-major); w8 = fp8 (W1,W2)
  pairs [128, 4, 2, 512]; w3 = bf16 W3 + a bias chunk [128, 5, 512].
- DMA pieces are ordered/sized against the cold DMA ramp (~40-130 GB/s for
  the first few us): smallest critical pieces first, weights for k-tile t
  land before its matmuls, w3 rides the sync queue between x pieces.
- DVE computes x^2/x^3 (bf16) per (tile, half); GpSimd casts x and x^2 to
  fp8 pairs. Both streams are order-pinned (Tile reorders otherwise).
- 64 matmuls: 32 DoubleRow fp8 + 32 bf16, into 8 PSUM banks (half, o-tile).
  Phase A (batch half 0) is t-major to pipeline with the x DMA; phase B
  (half 1) is o-major so each o-tile's output drains (descale+bias on DVE,
  DMA out) under the next tile's matmuls; the last o-tile drains in
  quarter-banks so only ~1us of work follows the final matmul.
- ~4.5us of dummy matmuls in the main block hold the PE clock gate at 8/8
  (2.4 GHz) before real work starts.
"""

import sys

import numpy as np
import ml_dtypes

sys.path.insert(0, "/opt/trn_rl_repo")

import concourse.bass as bass
import concourse.tile as tile
from concourse import bass_utils, mybir
from concourse.tile_rust import add_dep_helper

B, I, O = 8192, 512, 512
NCORES = 8
BS = B // NCORES  # 1024 rows per core
PT = 4  # 128-row tiles in I (k-tiles) and O (o-tiles)
NPOW = 3
SCALE = 256.0  # fp8 weight pre-scale (host); descaled in the output copy

BF = mybir.dt.bfloat16
F8 = mybir.dt.float8e4
F32 = mybir.dt.float32

_nc = None


def _build():
    # All HBM tensors are partition-major (leading dim 128 = SBUF partition).
    nc = bass.Bass("TRN2", target_bir_lowering=False, debug=False)
    xt = nc.dram_tensor("xt", [128, PT, BS], BF, kind="ExternalInput")
    w8 = nc.dram_tensor("w8", [128, PT, 2, O], F8, kind="ExternalInput")
    w3 = nc.dram_tensor("w3", [128, PT + 1, O], BF, kind="ExternalInput")
    yt = nc.dram_tensor("yt", [128, PT, BS], BF, kind="ExternalOutput")

    xt_r, w8_r, w3_r, yt_r = xt.ap(), w8.ap(), w3.ap(), yt.ap()

    chains = {}

    def chain(key, inst):
        prev = chains.get(key)
        if prev is not None:
            add_dep_helper(inst.ins, prev.ins, sync=False, reason=f"{key} order")
        chains[key] = inst
        return inst

    pe = lambda inst: chain("pe", inst)
    dve = lambda inst: chain("dve", inst)
    gp = lambda inst: chain("gp", inst)

    # HAM warm-up: ~4.5us of dummy matmuls on garbage SBUF in the main
    # block, so the PE clock gate is at 8/8 (2.4 GHz) when the real
    # matmuls start. Scratch PSUM is freed before the tile pools allocate;
    # real banks are zeroed by their start=True matmuls.
    warm_w = nc.alloc_sbuf_tensor("warm_w", [128, 2], BF)
    warm_x = nc.alloc_sbuf_tensor("warm_x", [128, 512], BF)
    with nc.psum_tensor("warm_ps", [128, 512], F32) as wps:
        for i in range(10):
            nc.tensor.matmul(
                wps.ap()[0:2, :], warm_w.ap(), warm_x.ap(), start=True, stop=True
            )
        # short tail warmups: fine-grained busy-keeping until real operands
        # land (insurance against HAM window phase drift)
        for i in range(8):
            nc.tensor.matmul(
                wps.ap()[0:2, 0:128],
                warm_w.ap(),
                warm_x.ap()[:, 0:128],
                start=True,
                stop=True,
            )

    with tile.TileContext(nc) as tc:
        with (
            tc.tile_pool(name="consts", bufs=1) as cpool,
            tc.tile_pool(name="acts", bufs=1) as apool,
            tc.tile_pool(name="out", bufs=1) as opool,
            tc.tile_pool(name="psum", bufs=1, space="PSUM") as pspool,
        ):
            w8_sb = cpool.tile([128, PT, 2, O], F8)
            w3_sb = cpool.tile([128, PT + 1, O], BF)
            x_sb = apool.tile([128, PT, BS], BF)
            x2_sb = apool.tile([128, PT, BS], BF)
            x3_sb = apool.tile([128, PT, BS], BF)
            b8_sb = apool.tile([128, PT, 2, BS], F8)  # (x, x^2) fp8 pairs
            y_sb = opool.tile([128, PT, BS], BF)

            # bias: w3 chunk 4, cols 0..7 hold [128,4] f32 as bf16 pairs
            bias_f32 = w3_sb[:, PT, 0:8].bitcast(F32)  # [128, 4]

            # DMA pieces in consumption order, sized for the cold ramp.
            # sync: x half-0 tiles (small, they gate the pipeline) with w3
            # pieces in the gaps; x half-1 behind. scalar: fp8 weight pairs
            # per k-tile (each 128KB).
            h0, h1 = slice(0, 512), slice(512, 1024)
            nc.sync.dma_start(out=x_sb[:, 0, h0], in_=xt_r[:, 0, h0])
            nc.sync.dma_start(out=x_sb[:, 1, h0], in_=xt_r[:, 1, h0])
            nc.sync.dma_start(out=w3_sb[:, 0:1, :], in_=w3_r[:, 0:1, :])
            nc.sync.dma_start(out=w3_sb[:, 1:2, :], in_=w3_r[:, 1:2, :])
            nc.sync.dma_start(out=x_sb[:, 2, h0], in_=xt_r[:, 2, h0])
            nc.sync.dma_start(out=x_sb[:, 3, h0], in_=xt_r[:, 3, h0])
            nc.sync.dma_start(out=x_sb[:, 0:2, h1], in_=xt_r[:, 0:2, h1])
            nc.sync.dma_start(out=x_sb[:, 2:PT, h1], in_=xt_r[:, 2:PT, h1])
            # scalar queue: fp8 weight DMAs + the w3 tail (its transfer
            # overlaps the scalar casts; only the issue takes engine time)
            for t in range(PT):
                nc.scalar.dma_start(out=w8_sb[:, t, :, :], in_=w8_r[:, t, :, :])
            nc.scalar.dma_start(
                out=w3_sb[:, 2 : PT + 1, :], in_=w3_r[:, 2 : PT + 1, :]
            )

            # DVE: squares/cubes per (tile, half) as x lands.
            # Scalar engine: fp8 casts of x and x^2 into the pair tensor
            # (gpsimd is far too slow for these; scalar's queue is free
            # after its 4 weight-DMA issues).
            Copy = mybir.ActivationFunctionType.Copy
            # half 0 (phase A): DVE does x^2/x^3 per tile; scalar casts
            # both fp8 pair slots.
            for t in range(PT):
                dve(nc.vector.tensor_mul(x2_sb[:, t, h0], x_sb[:, t, h0], x_sb[:, t, h0]))
                dve(nc.vector.tensor_mul(x3_sb[:, t, h0], x2_sb[:, t, h0], x_sb[:, t, h0]))
                gp(nc.scalar.activation(b8_sb[:, t, 0, h0], x_sb[:, t, h0], Copy))
                gp(nc.scalar.activation(b8_sb[:, t, 1, h0], x2_sb[:, t, h0], Copy))
            # half 1 (phase B): fp8 pair products first (phase B runs all
            # DoubleRow matmuls before any x^3 matmul), x^3 after. x^2
            # casts on DVE, x casts on scalar.
            for t in range(2):
                dve(nc.vector.tensor_mul(x2_sb[:, t, h1], x_sb[:, t, h1], x_sb[:, t, h1]))
                dve(nc.vector.tensor_copy(b8_sb[:, t, 1, h1], x2_sb[:, t, h1]))
            for t in range(2, PT):
                dve(nc.vector.tensor_mul(x2_sb[:, t, h1], x_sb[:, t, h1], x_sb[:, t, h1]))
                dve(nc.vector.tensor_copy(b8_sb[:, t, 1, h1], x2_sb[:, t, h1]))
            for t in range(PT):
                dve(nc.vector.tensor_mul(x3_sb[:, t, h1], x2_sb[:, t, h1], x_sb[:, t, h1]))
            for t in range(PT):
                gp(nc.scalar.activation(b8_sb[:, t, 0, h1], x_sb[:, t, h1], Copy))

            # 8 PSUM banks: ps[n*4+o] = batch half n, o-tile o, [128, 512] f32
            ps = [
                pspool.tile([128, 512], F32, name=f"ps{i}", tag=f"ps{i}")
                for i in range(2 * PT)
            ]

            def ncols(n):
                return slice(n * 512, (n + 1) * 512)

            def osl(o):
                return slice(o * 128, (o + 1) * 128)

            def mm_dr(n, o, t, start=False, stop=False):
                # fp8 DoubleRow: contracts the (W1,W2)x(x,x^2) pair (256 deep)
                pe(
                    nc.tensor.matmul(
                        ps[n * PT + o],
                        w8_sb[:, t, :, osl(o)],
                        b8_sb[:, t, :, ncols(n)],
                        start=start,
                        stop=stop,
                        perf_mode=mybir.MatmulPerfMode.DoubleRow,
                    )
                )

            def mm_p3(n, o, t, start=False, stop=False):
                pe(
                    nc.tensor.matmul(
                        ps[n * PT + o],
                        w3_sb[:, t, osl(o)],
                        x3_sb[:, t, ncols(n)],
                        start=start,
                        stop=stop,
                    )
                )

            def copy_out(n, o, cs, bank_cs):
                # PSUM -> SBUF: descale (1/S) then add bias (f32 -> bf16)
                dve(
                    nc.vector.tensor_scalar(
                        y_sb[:, o, cs],
                        ps[n * PT + o][:, bank_cs],
                        1.0 / SCALE,
                        bias_f32[:, o : o + 1],
                        op0=mybir.AluOpType.mult,
                        op1=mybir.AluOpType.add,
                    )
                )

            # phase A: batch half 0, t-major (pipelines with x arrival).
            # x^3 matmuls lead each tile (w3 + DVE x^3 land before the fp8
            # casts), DoubleRow follows.
            for t in range(PT):
                for o in range(PT):
                    mm_p3(0, o, t, start=(t == 0))
                for o in range(PT):
                    mm_dr(0, o, t, stop=(t == PT - 1))
            # phase B: batch half 1. All DoubleRow matmuls first, t-major
            # (their fp8 pairs are ready earliest), then per-o x^3 groups
            # with overlapped drains; o3's half-0 output ships early so
            # only a quarter-bank drain follows the very last matmul.
            for o in range(PT):
                copy_out(0, o, h0, slice(0, 512))
            nc.scalar.dma_start(out=yt_r[:, 3, h0], in_=y_sb[:, 3, h0])
            for t in range(PT):
                for o in range(PT):
                    mm_dr(1, o, t, start=(t == 0))
            for o in range(PT):
                for t in range(PT):
                    mm_p3(1, o, t, stop=(t == PT - 1))
                if o < 3:
                    copy_out(1, o, h1, slice(0, 512))
                    queue = nc.scalar if o % 2 == 0 else nc.sync
                    queue.dma_start(out=yt_r[:, o, :], in_=y_sb[:, o, :])
                else:
                    for q in range(2):
                        cs = slice(512 + q * 256, 768 + q * 256)
                        copy_out(1, 3, cs, slice(q * 256, (q + 1) * 256))
                        queue = nc.sync if q == 0 else nc.scalar
                        queue.dma_start(out=yt_r[:, 3, cs], in_=y_sb[:, 3, cs])

    # Post-pass: walrus codegen admits only one sync-wait per instruction
    # encoding here; split any multi-wait instruction into a chain of
    # single-wait drains ahead of it on the same engine queue.
    for bb in nc.m.functions[0].blocks:
        insts = list(bb.instructions)
        out, split = [], 0
        for ins in insts:
            si = ins.sync_info
            waits = list(si.on_wait) if si and si.on_wait else []
            if len(waits) > 1:
                for wx in waits[:-1]:
                    nd = mybir.InstDrain(
                        name=f"drain_split_{split}", engine=ins.engine
                    )
                    split += 1
                    nd.sync_info = mybir.SyncInfo(on_wait=[wx], on_update=[])
                    out.append(nd)
                si.on_wait = [waits[-1]]
            out.append(ins)
        if split:
            bb.set_instructions_from_list(out) if hasattr(
                bb, "set_instructions_from_list"
            ) else setattr(bb, "instructions", out)
    return nc


last_results = None  # BassKernelResults of the most recent run (for test harness)


def kernel(x, edge_w, edge_b, comb_w):
    global _nc, last_results
    if _nc is None:
        _nc = _build()

    bf16 = ml_dtypes.bfloat16
    f8 = ml_dtypes.float8_e4m3
    w_eff = (edge_w * comb_w[:, :, None]).astype(np.float32)  # [I, O, 3]

    # fp8 pair weights [128, 4, 2, 512]: [q, t, pair p, o] = S*Wp[t*128+q, o]
    w8_pm = np.empty((128, PT, 2, O), dtype=f8)
    # bf16 W3 + bias chunk [128, 5, 512]
    w3_pm = np.empty((128, PT + 1, O), dtype=bf16)
    for t in range(PT):
        rows = slice(t * 128, (t + 1) * 128)
        for p in range(2):
            w8_pm[:, t, p, :] = (SCALE * w_eff[rows, :, p]).astype(f8)
        w3_pm[:, t, :] = (SCALE * w_eff[rows, :, 2]).astype(bf16)
    # bias chunk: [128,4] f32 bit-packed into bf16 pairs at cols 0..7
    bias = np.sum(comb_w * edge_b, axis=0, dtype=np.float64).astype(np.float32)
    pad = np.zeros((128, O), dtype=bf16)
    pad.view(np.uint16)[:, :8] = np.ascontiguousarray(
        bias.reshape(PT, 128).T
    ).view(np.uint16)
    w3_pm[:, PT, :] = pad
    w8_pm = np.ascontiguousarray(w8_pm)
    w3_pm = np.ascontiguousarray(w3_pm)

    in_maps = []
    for c in range(NCORES):
        xs = x[c * BS : (c + 1) * BS].T.astype(bf16)  # [I, BS]
        # partition-major: [128, 4, 1024], [p, t, b] = x^T[t*128+p, b]
        xs_pm = np.ascontiguousarray(xs.reshape(PT, 128, BS).transpose(1, 0, 2))
        in_maps.append({"xt": xs_pm, "w8": w8_pm, "w3": w3_pm})

    res = bass_utils.run_bass_kernel_spmd(_nc, in_maps, list(range(NCORES)))
    last_results = res
    outs = []
    for c in range(NCORES):
        yt = np.asarray(res.results[c]["yt"])  # [128, 4, 1024] bf16
        outs.append(yt.transpose(1, 0, 2).reshape(O, BS).T.astype(np.float32))
    return np.concatenate(outs, axis=0)


# revision 30
# speedup vs baseline: 1.0157x; 1.0157x over previous
"""KAN layer kernel for Trainium2, data-parallel over 8 NeuronCores.

Math: out[b,o] = sum_i comb_w[i,o] * (w1*x + w2*x^2 + w3*x^3 + edge_b)[b,i,o]
    = x @ W1 + x^2 @ W2 + x^3 @ W3 + bias
  where Wp[i,o] = edge_w[i,o,p] * comb_w[i,o],  bias[o] = sum_i comb_w[i,o]*edge_b[i,o].

Sharding: batch 8-way (1024 rows/core), weights replicated.

Precision: hybrid. The x and x^2 terms (2/3 of the contraction, small share
of the output error) run in fp8-e4m3 DoubleRow matmuls — each instruction
contracts 256 (a (W1,W2) weight pair against an (x, x^2) basis pair) so the
x/x^2 work needs half the instructions of bf16. The error-dominant x^3 term
stays bf16. All weights are pre-scaled by S=256 on the host so fp8 values
sit in e4m3's sweet spot; the PSUM result is descaled by 1/S during the
bias-add copy. Verified numerics: max rel err 1.13e-2 vs the fp32 reference
(gate is 2e-2). PSUM accumulates fp32 throughout.

Per core:
- x^T arrives as [128, 4, 1024] bf16 (partition-major); w8 = fp8 (W1,W2)
  pairs [128, 4, 2, 512]; w3 = bf16 W3 + a bias chunk [128, 5, 512].
- DMA pieces are ordered/sized against the cold DMA ramp (~40-130 GB/s for
  the first few us): smallest critical pieces first, weights for k-tile t
  land before its matmuls, w3 rides the sync queue between x pieces.
- DVE computes x^2/x^3 (bf16) per (tile, half); GpSimd casts x and x^2 to
  fp8 pairs. Both streams are order-pinned (Tile reorders otherwise).
- 64 matmuls: 32 DoubleRow fp8 + 32 bf16, into 8 PSUM banks (half, o-tile).
  Phase A (batch half 0) is t-major to pipeline with the x DMA; phase B
  (half 1) is o-major so each o-tile's output drains (descale+bias on DVE,
  DMA out) under the next tile's matmuls; the last o-tile drains in
  quarter-banks so only ~1us of work follows the final matmul.
- ~4.5us of dummy matmuls in the main block hold the PE clock gate at 8/8
  (2.4 GHz) before real work starts.
"""

import sys

import numpy as np
import ml_dtypes

sys.path.insert(0, "/opt/trn_rl_repo")

import concourse.bass as bass
import concourse.tile as tile
from concourse import bass_utils, mybir
from concourse.tile_rust import add_dep_helper

B, I, O = 8192, 512, 512
NCORES = 8
BS = B // NCORES  # 1024 rows per core
PT = 4  # 128-row tiles in I (k-tiles) and O (o-tiles)
NPOW = 3
SCALE = 256.0  # fp8 weight pre-scale (host); descaled in the output copy

BF = mybir.dt.bfloat16
F8 = mybir.dt.float8e4
F32 = mybir.dt.float32

_nc = None


def _build():
    # All HBM tensors are partition-major (leading dim 128 = SBUF partition).
    nc = bass.Bass("TRN2", target_bir_lowering=False, debug=False)
    xt = nc.dram_tensor("xt", [128, PT, BS], BF, kind="ExternalInput")
    w8 = nc.dram_tensor("w8", [128, PT, 2, O], F8, kind="ExternalInput")
    w3 = nc.dram_tensor("w3", [128, PT + 1, O], BF, kind="ExternalInput")
    yt = nc.dram_tensor("yt", [128, PT, BS], BF, kind="ExternalOutput")

    xt_r, w8_r, w3_r, yt_r = xt.ap(), w8.ap(), w3.ap(), yt.ap()

    chains = {}

    def chain(key, inst):
        prev = chains.get(key)
        if prev is not None:
            add_dep_helper(inst.ins, prev.ins, sync=False, reason=f"{key} order")
        chains[key] = inst
        return inst

    pe = lambda inst: chain("pe", inst)
    dve = lambda inst: chain("dve", inst)
    gp = lambda inst: chain("gp", inst)

    # HAM warm-up: ~4.5us of dummy matmuls on garbage SBUF in the main
    # block, so the PE clock gate is at 8/8 (2.4 GHz) when the real
    # matmuls start. Scratch PSUM is freed before the tile pools allocate;
    # real banks are zeroed by their start=True matmuls.
    warm_w = nc.alloc_sbuf_tensor("warm_w", [128, 2], BF)
    warm_x = nc.alloc_sbuf_tensor("warm_x", [128, 512], BF)
    with nc.psum_tensor("warm_ps", [128, 512], F32) as wps:
        for i in range(10):
            nc.tensor.matmul(
                wps.ap()[0:2, :], warm_w.ap(), warm_x.ap(), start=True, stop=True
            )
        # short tail warmups: fine-grained busy-keeping until real operands
        # land (insurance against HAM window phase drift)
        for i in range(8):
            nc.tensor.matmul(
                wps.ap()[0:2, 0:128],
                warm_w.ap(),
                warm_x.ap()[:, 0:128],
                start=True,
                stop=True,
            )

    with tile.TileContext(nc) as tc:
        with (
            tc.tile_pool(name="consts", bufs=1) as cpool,
            tc.tile_pool(name="acts", bufs=1) as apool,
            tc.tile_pool(name="out", bufs=1) as opool,
            tc.tile_pool(name="psum", bufs=1, space="PSUM") as pspool,
        ):
            w8_sb = cpool.tile([128, PT, 2, O], F8)
            w3_sb = cpool.tile([128, PT + 1, O], BF)
            x_sb = apool.tile([128, PT, BS], BF)
            x2_sb = apool.tile([128, PT, BS], BF)
            x3_sb = apool.tile([128, PT, BS], BF)
            b8_sb = apool.tile([128, PT, 2, BS], F8)  # (x, x^2) fp8 pairs
            y_sb = opool.tile([128, PT, BS], BF)

            # bias: w3 chunk 4, cols 0..7 hold [128,4] f32 as bf16 pairs
            bias_f32 = w3_sb[:, PT, 0:8].bitcast(F32)  # [128, 4]

            # DMA pieces in consumption order, sized for the cold ramp.
            # sync: x half-0 tiles (small, they gate the pipeline) with w3
            # pieces in the gaps; x half-1 behind. scalar: fp8 weight pairs
            # per k-tile (each 128KB).
            h0, h1 = slice(0, 512), slice(512, 1024)
            nc.sync.dma_start(out=x_sb[:, 0, h0], in_=xt_r[:, 0, h0])
            nc.sync.dma_start(out=x_sb[:, 1, h0], in_=xt_r[:, 1, h0])
            nc.sync.dma_start(out=w3_sb[:, 0:1, :], in_=w3_r[:, 0:1, :])
            nc.sync.dma_start(out=w3_sb[:, 1:2, :], in_=w3_r[:, 1:2, :])
            nc.sync.dma_start(out=x_sb[:, 2, h0], in_=xt_r[:, 2, h0])
            nc.sync.dma_start(out=x_sb[:, 3, h0], in_=xt_r[:, 3, h0])
            nc.sync.dma_start(out=x_sb[:, 0:2, h1], in_=xt_r[:, 0:2, h1])
            nc.sync.dma_start(out=x_sb[:, 2:PT, h1], in_=xt_r[:, 2:PT, h1])
            # scalar queue: fp8 weight DMAs + the w3 tail (its transfer
            # overlaps the scalar casts; only the issue takes engine time)
            for t in range(PT):
                nc.scalar.dma_start(out=w8_sb[:, t, :, :], in_=w8_r[:, t, :, :])
            nc.scalar.dma_start(
                out=w3_sb[:, 2 : PT + 1, :], in_=w3_r[:, 2 : PT + 1, :]
            )

            # DVE: squares/cubes per (tile, half) as x lands.
            # Scalar engine: fp8 casts of x and x^2 into the pair tensor
            # (gpsimd is far too slow for these; scalar's queue is free
            # after its 4 weight-DMA issues).
            Copy = mybir.ActivationFunctionType.Copy
            # DVE owns all x^2 products and their fp8 casts (it has slack
            # in both phases); the scalar engine only casts x itself, so
            # its slower pipeline never gates a DoubleRow matmul.
            # half 0 (phase A), per tile: x^2, its cast, x^3.
            for t in range(PT):
                dve(nc.vector.tensor_mul(x2_sb[:, t, h0], x_sb[:, t, h0], x_sb[:, t, h0]))
                dve(nc.vector.tensor_copy(b8_sb[:, t, 1, h0], x2_sb[:, t, h0]))
                dve(nc.vector.tensor_mul(x3_sb[:, t, h0], x2_sb[:, t, h0], x_sb[:, t, h0]))
            # half 1 (phase B): all fp8 pair products first (phase B runs
            # every DoubleRow matmul before any x^3 matmul), x^3 after.
            for t in range(PT):
                dve(nc.vector.tensor_mul(x2_sb[:, t, h1], x_sb[:, t, h1], x_sb[:, t, h1]))
                dve(nc.vector.tensor_copy(b8_sb[:, t, 1, h1], x2_sb[:, t, h1]))
            for t in range(PT):
                dve(nc.vector.tensor_mul(x3_sb[:, t, h1], x2_sb[:, t, h1], x_sb[:, t, h1]))
            for t in range(PT):
                gp(nc.scalar.activation(b8_sb[:, t, 0, h0], x_sb[:, t, h0], Copy))
            for t in range(PT):
                gp(nc.scalar.activation(b8_sb[:, t, 0, h1], x_sb[:, t, h1], Copy))

            # 8 PSUM banks: ps[n*4+o] = batch half n, o-tile o, [128, 512] f32
            ps = [
                pspool.tile([128, 512], F32, name=f"ps{i}", tag=f"ps{i}")
                for i in range(2 * PT)
            ]

            def ncols(n):
                return slice(n * 512, (n + 1) * 512)

            def osl(o):
                return slice(o * 128, (o + 1) * 128)

            def mm_dr(n, o, t, start=False, stop=False):
                # fp8 DoubleRow: contracts the (W1,W2)x(x,x^2) pair (256 deep)
                pe(
                    nc.tensor.matmul(
                        ps[n * PT + o],
                        w8_sb[:, t, :, osl(o)],
                        b8_sb[:, t, :, ncols(n)],
                        start=start,
                        stop=stop,
                        perf_mode=mybir.MatmulPerfMode.DoubleRow,
                    )
                )

            def mm_p3(n, o, t, start=False, stop=False):
                pe(
                    nc.tensor.matmul(
                        ps[n * PT + o],
                        w3_sb[:, t, osl(o)],
                        x3_sb[:, t, ncols(n)],
                        start=start,
                        stop=stop,
                    )
                )

            def copy_out(n, o, cs, bank_cs):
                # PSUM -> SBUF: descale (1/S) then add bias (f32 -> bf16)
                dve(
                    nc.vector.tensor_scalar(
                        y_sb[:, o, cs],
                        ps[n * PT + o][:, bank_cs],
                        1.0 / SCALE,
                        bias_f32[:, o : o + 1],
                        op0=mybir.AluOpType.mult,
                        op1=mybir.AluOpType.add,
                    )
                )

            # phase A: batch half 0, t-major (pipelines with x arrival).
            # x^3 matmuls lead each tile (w3 + DVE x^3 land before the fp8
            # casts), DoubleRow follows.
            for t in range(PT):
                for o in range(PT):
                    mm_p3(0, o, t, start=(t == 0))
                for o in range(PT):
                    mm_dr(0, o, t, stop=(t == PT - 1))
            # phase B: batch half 1. All DoubleRow matmuls first, t-major
            # (their fp8 pairs are ready earliest), then per-o x^3 groups
            # with overlapped drains; o3's half-0 output ships early so
            # only a quarter-bank drain follows the very last matmul.
            for o in range(PT):
                copy_out(0, o, h0, slice(0, 512))
            nc.scalar.dma_start(out=yt_r[:, 3, h0], in_=y_sb[:, 3, h0])
            for t in range(PT):
                for o in range(PT):
                    mm_dr(1, o, t, start=(t == 0))
            for o in range(PT):
                for t in range(PT):
                    mm_p3(1, o, t, stop=(t == PT - 1))
                if o < 3:
                    copy_out(1, o, h1, slice(0, 512))
                    queue = nc.scalar if o % 2 == 0 else nc.sync
                    queue.dma_start(out=yt_r[:, o, :], in_=y_sb[:, o, :])
                else:
                    for q in range(2):
                        cs = slice(512 + q * 256, 768 + q * 256)
                        copy_out(1, 3, cs, slice(q * 256, (q + 1) * 256))
                        queue = nc.sync if q == 0 else nc.scalar
                        queue.dma_start(out=yt_r[:, 3, cs], in_=y_sb[:, 3, cs])

    # Post-pass: walrus codegen admits only one sync-wait per instruction
    # encoding here; split any multi-wait instruction into a chain of
    # single-wait drains ahead of it on the same engine queue.
    for bb in nc.m.functions[0].blocks:
        insts = list(bb.instructions)
        out, split = [], 0
        for ins in insts:
            si = ins.sync_info
            waits = list(si.on_wait) if si and si.on_wait else []
            if len(waits) > 1:
                for wx in waits[:-1]:
                    nd = mybir.InstDrain(
                        name=f"drain_split_{split}", engine=ins.engine
                    )
                    split += 1
                    nd.sync_info = mybir.SyncInfo(on_wait=[wx], on_update=[])
                    out.append(nd)
                si.on_wait = [waits[-1]]
            out.append(ins)
        if split:
            bb.set_instructions_from_list(out) if hasattr(
                bb, "set_instructions_from_list"
            ) else setattr(bb, "instructions", out)
    return nc


last_results = None  # BassKernelResults of the most recent run (for test harness)


def kernel(x, edge_w, edge_b, comb_w):
    global _nc, last_results
    if _nc is None:
        _nc = _build()

    bf16 = ml_dtypes.bfloat16
    f8 = ml_dtypes.float8_e4m3
    w_eff = (edge_w * comb_w[:, :, None]).astype(np.float32)  # [I, O, 3]

    # fp8 pair weights [128, 4, 2, 512]: [q, t, pair p, o] = S*Wp[t*128+q, o]
    w8_pm = np.empty((128, PT, 2, O), dtype=f8)
    # bf16 W3 + bias chunk [128, 5, 512]
    w3_pm = np.empty((128, PT + 1, O), dtype=bf16)
    for t in range(PT):
        rows = slice(t * 128, (t + 1) * 128)
        for p in range(2):
            w8_pm[:, t, p, :] = (SCALE * w_eff[rows, :, p]).astype(f8)
        w3_pm[:, t, :] = (SCALE * w_eff[rows, :, 2]).astype(bf16)
    # bias chunk: [128,4] f32 bit-packed into bf16 pairs at cols 0..7
    bias = np.sum(comb_w * edge_b, axis=0, dtype=np.float64).astype(np.float32)
    pad = np.zeros((128, O), dtype=bf16)
    pad.view(np.uint16)[:, :8] = np.ascontiguousarray(
        bias.reshape(PT, 128).T
    ).view(np.uint16)
    w3_pm[:, PT, :] = pad
    w8_pm = np.ascontiguousarray(w8_pm)
    w3_pm = np.ascontiguousarray(w3_pm)

    in_maps = []
    for c in range(NCORES):
        xs = x[c * BS : (c + 1) * BS].T.astype(bf16)  # [I, BS]
        # partition-major: [128, 4, 1024], [p, t, b] = x^T[t*128+p, b]
        xs_pm = np.ascontiguousarray(xs.reshape(PT, 128, BS).transpose(1, 0, 2))
        in_maps.append({"xt": xs_pm, "w8": w8_pm, "w3": w3_pm})

    res = bass_utils.run_bass_kernel_spmd(_nc, in_maps, list(range(NCORES)))
    last_results = res
    outs = []
    for c in range(NCORES):
        yt = np.asarray(res.results[c]["yt"])  # [128, 4, 1024] bf16
        outs.append(yt.transpose(1, 0, 2).reshape(O, BS).T.astype(np.float32))
    return np.concatenate(outs, axis=0)
